# revision 25
# baseline (speedup 1.0000x reference)
"""Trainium2 Bass kernel for CompositionalPINN forward.

Reference semantics (B=262144, H=256, N_STEPS=8):
    state = state_dz[:, :4]; qop = state_dz[:, 4:5]; dz_sub = state_dz[:, 5:6]/8
    n_full = floor(z_frac*8); frac = z_frac*8 - n_full
    for step in range(8):
        state += (n_full > step) * MLP(state, qop, dz_sub)        # residual MLP
    state += (frac > 1e-6) * MLP(state, qop, frac*dz_sub)
    MLP(x) = silu(silu(silu(x@W1+b1)@W2+b2)@W3+b3)@W4+b4  (6->256->256->256->4)

Strategy: pure data parallel over 8 cores.  Host transposes inputs to a
feature-major layout and sorts samples by n_full (descending, dealt
round-robin across cores) so each 512-sample tile only runs
max(n_full)+1 MLP evals instead of 9 (data lower bound ~4.5 avg).

Per-core schedule is baked into the compiled program (cached in-process
by schedule signature).

Engine budget per tile-eval (512 samples): PE 12 matmuls x 512 cols
~2.6us warm, ACT 3 silus on [128,1024] ~3.3us, DVE ~1.5us.  ACT is the
floor; the kernel keeps all three engines streaming:

  - Emission is LAYER-interleaved across GROUP=4 tiles per round
    (L1 x4 -> silu1 x4 -> L2 x4 -> ...), so the PE never head-of-line
    blocks on a silu and stays HAM-warm (2.4 GHz), and ACT/DVE stream.
  - All matmul operands bf16: 2-byte LDWEIGHTS + automatic fast weight
    load, hidden behind matmuls by the PE reorder window.  PSUM stays
    f32; the f32 state accumulator lives in SBUF.
  - Masked (boundary-tile) evals: host packs a 4-row f32 mask per
    masked eval, DMA'd into SBUF just-in-time; applied as one DVE
    multiply on the [4,512] delta.  No PE mask-broadcast matmul.
  - The bf16 [8, 512] matmul inputs live in per-chunk staging tiles:
    the host pre-packs rows as [bf16(state), qop, dz_sub, dz_part, 0]
    and one DMA per 8-tile chunk fills them (engines cannot write at
    partition base 4, DMA can).  After each eval, one fused DVE
    tensor_add writes bf16(state+delta) into rows 0-3 for the next
    round, off the critical path; the f32 state accumulator in SBUF is
    updated separately.

PSUM: 3 x [128,1024] h-tiles (2 banks each) + 2 x [4,512] delta = 8 banks.
"""

import numpy as np
from contextlib import ExitStack

import concourse.bass as bass
import concourse.tile as tile
from concourse import bacc, mybir
from concourse.bass_utils import run_bass_kernel_spmd

F32 = mybir.dt.float32
BF16 = mybir.dt.bfloat16
Silu = mybir.ActivationFunctionType.Silu

NCORES = 8
NTILE = 512
CHUNK_TILES = 4                     # tiles per DMA chunk
GROUP = 4                           # tiles interleaved per round
H = 256
NSTEPS = 8


def _chunks(tiles):
    return [(c0, min(c0 + CHUNK_TILES, tiles)) for c0 in range(0, tiles, CHUNK_TILES)]


def _emit_order(schedule, tiles):
    """Yield (tile_index, eval_desc) in device emission order.  Used by both
    the program builder and the host maskcat packer — must stay identical."""
    for (c0, c1) in _chunks(tiles):
        for g0 in range(c0, c1, GROUP):
            group = list(range(g0, min(g0 + GROUP, c1)))
            maxev = max((len(schedule[t]) for t in group), default=0)
            for r in range(maxev):
                for t in group:
                    if r < len(schedule[t]):
                        yield t, schedule[t][r]


_BUILD_CACHE = {}

LAST_EXEC_NS = None  # set when BASSK_TRACE=1


def _install_ntff_hook():
    """The agent image lacks antenv.axon_hooks; synthesize it so
    run_bass_kernel_spmd(trace=True) can reach the NTFF profiler."""
    import sys
    import types
    if "antenv.axon_hooks" in sys.modules:
        return True
    try:
        import antenv
        from trn_agent_boot.trn_boot import _ntff_profile_via_ctypes
        hook = _ntff_profile_via_ctypes("/opt/axon/libaxon_pjrt.so")
        if hook is None:
            return False
        mod = types.ModuleType("antenv.axon_hooks")
        mod.get_axon_ntff_profile_hook = lambda: hook
        mod.set_axon_ntff_profile_hook = lambda h: None
        sys.modules["antenv.axon_hooks"] = mod
        antenv.axon_hooks = mod
        return True
    except Exception:
        return False


def _build(schedule, use_bias, n_core):
    """schedule: tuple over tiles of tuples of (is_partial, use_mask)."""
    tiles = n_core // NTILE
    n_masked = sum(1 for tev in schedule for (_, m) in tev if m)
    nc = bacc.Bacc("TRN2", target_bir_lowering=False, debug=False,
                   num_devices=NCORES)

    xs_d = nc.declare_dram_parameter("xs", [4, n_core], F32, isOutput=False)
    st_d = nc.declare_dram_parameter("stat", [8, n_core], BF16, isOutput=False)
    mk_d = nc.declare_dram_parameter("maskcat", [4, max(1, n_masked) * NTILE],
                                     F32, isOutput=False)
    w1_d = nc.declare_dram_parameter("w1", [8, 512], BF16, isOutput=False)
    w2_d = nc.declare_dram_parameter("w2", [128, 512], BF16, isOutput=False)
    w3_d = nc.declare_dram_parameter("w3", [128, 512], BF16, isOutput=False)
    w4_d = nc.declare_dram_parameter("w4", [128, 8], BF16, isOutput=False)
    if use_bias:
        b123_d = nc.declare_dram_parameter("b123", [128, 6], F32, isOutput=False)
        b4_d = nc.declare_dram_parameter("b4r", [4, 1], F32, isOutput=False)
    out_d = nc.declare_dram_parameter("outT", [4, n_core], F32, isOutput=True)

    chunks = _chunks(tiles)

    with tile.TileContext(nc) as tc, ExitStack() as ctx:
        const = ctx.enter_context(tc.tile_pool(name="const", bufs=1))
        data = ctx.enter_context(tc.tile_pool(name="data", bufs=1))
        stgp = ctx.enter_context(tc.tile_pool(name="stg", bufs=3))
        acts = ctx.enter_context(tc.tile_pool(name="acts", bufs=8))
        mskp = ctx.enter_context(tc.tile_pool(name="msk", bufs=4))
        tmpp = ctx.enter_context(tc.tile_pool(name="tmp", bufs=2))
        ps_h = ctx.enter_context(tc.tile_pool(name="ps_h", bufs=3, space="PSUM"))
        ps_d = ctx.enter_context(tc.tile_pool(name="ps_d", bufs=2, space="PSUM"))

        # ---- prime the ACT spline table during the DMA warmup window
        dumm = const.tile([1, 2], F32)
        nc.vector.memset(dumm, 0.0)
        nc.scalar.activation(dumm[:, 1:2], dumm[:, 0:1], Silu)

        # ---- weights: DMA bf16 directly (host pre-casts); spread across
        # queues so w2/w3 (128KB each) land before round 0 reaches L2/L3
        w1 = const.tile([8, 512], BF16)
        w2 = const.tile([128, 512], BF16)
        nc.sync.dma_start(out=w2, in_=w2_d[:, :])
        w3 = const.tile([128, 512], BF16)
        nc.scalar.dma_start(out=w3, in_=w3_d[:, :])
        w4 = const.tile([128, 8], BF16)
        if use_bias:
            b123 = const.tile([128, 6], F32)
            nc.gpsimd.dma_start(out=b123, in_=b123_d[:, :])
            b4r = const.tile([4, 1], F32)
            nc.gpsimd.dma_start(out=b4r, in_=b4_d[:, :])

        # ---- f32 state accumulator stays resident in SBUF
        xs = data.tile([4, n_core], F32)
        for (c0, c1) in chunks:
            nc.sync.dma_start(out=xs[:, c0 * NTILE:c1 * NTILE],
                              in_=xs_d[:, c0 * NTILE:c1 * NTILE])
        stg_tiles = {}

        def fetch_stg(ci):
            if ci in stg_tiles or ci >= len(chunks):
                return
            cc0, cc1 = chunks[ci]
            t_ = stgp.tile([8, (cc1 - cc0) * NTILE], BF16, tag="stg")
            nc.gpsimd.dma_start(out=t_, in_=st_d[:, cc0 * NTILE:cc1 * NTILE])
            stg_tiles[ci] = t_

        # round-0 critical path: stg chunk 0 first on the gpsimd queue,
        # then the small L1/L4 weights
        fetch_stg(0)
        nc.gpsimd.dma_start(out=w1, in_=w1_d[:, :])
        nc.gpsimd.dma_start(out=w4, in_=w4_d[:, :])

        mask_slot = [0]

        def silu_phase(hp, bias_idx):
            hs = acts.tile([128, 2 * NTILE], BF16, tag="h")
            if use_bias:
                nc.scalar.activation(hs[:, 0:NTILE], hp[:, 0:NTILE], Silu,
                                     bias=b123[:, bias_idx:bias_idx + 1])
                nc.scalar.activation(hs[:, NTILE:], hp[:, NTILE:], Silu,
                                     bias=b123[:, bias_idx + 1:bias_idx + 2])
            else:
                nc.scalar.activation(hs, hp, Silu)
            return hs

        def mid_layer(hs, w):
            hp = ps_h.tile([128, 2 * NTILE], F32, tag="h")
            for mt in range(2):
                for kt in range(2):
                    nc.tensor.matmul(
                        hp[:, mt * NTILE:(mt + 1) * NTILE],
                        w[:, kt * 256 + mt * 128: kt * 256 + (mt + 1) * 128],
                        hs[:, kt * NTILE:(kt + 1) * NTILE],
                        start=(kt == 0), stop=(kt == 1))
            return hp

        for ci, (c0, c1) in enumerate(chunks):
            stg = stg_tiles[ci]
            def emit_l1(t, r):
                w1off = 256 if schedule[t][r][0] else 0
                xt = stg[:, (t - c0) * NTILE:(t - c0 + 1) * NTILE]
                h1p = ps_h.tile([128, 2 * NTILE], F32, tag="h")
                nc.tensor.matmul(h1p[:, 0:NTILE], w1[:, w1off:w1off + 128],
                                 xt, start=True, stop=True)
                nc.tensor.matmul(h1p[:, NTILE:2 * NTILE],
                                 w1[:, w1off + 128:w1off + 256],
                                 xt, start=True, stop=True)
                return h1p

            for g0 in range(c0, c1, GROUP):
                group = list(range(g0, min(g0 + GROUP, c1)))
                maxev = max((len(schedule[t]) for t in group), default=0)
                h1ps = {}
                for r in range(maxev):
                    active = [t for t in group if r < len(schedule[t])]
                    # mask DMAs for this round (prefetch ~8us ahead of use)
                    mks = {}
                    for t in active:
                        if schedule[t][r][1]:
                            mk = mskp.tile([4, NTILE], F32, tag="mk")
                            j = mask_slot[0]
                            mask_slot[0] += 1
                            nc.gpsimd.dma_start(
                                out=mk, in_=mk_d[:, j * NTILE:(j + 1) * NTILE])
                            mks[t] = mk
                    # L1: round 0 here; later rounds pre-issued at the
                    # previous round's apply step so ACT never waits
                    for t in active:
                        if t not in h1ps:
                            h1ps[t] = emit_l1(t, r)
                    h1ss = {t: silu_phase(h1ps.pop(t), 0) for t in active}
                    h2ps = {t: mid_layer(h1ss[t], w2) for t in active}
                    h2ss = {t: silu_phase(h2ps[t], 2) for t in active}
                    h3ps = {t: mid_layer(h2ss[t], w3) for t in active}
                    h3ss = {t: silu_phase(h3ps[t], 4) for t in active}
                    # L4 phase
                    dd = {}
                    for t in active:
                        d = ps_d.tile([4, NTILE], F32, tag="d")
                        nc.tensor.matmul(d, w4[:, 0:4], h3ss[t][:, 0:NTILE],
                                         start=True, stop=False)
                        nc.tensor.matmul(d, w4[:, 4:8], h3ss[t][:, NTILE:2 * NTILE],
                                         start=False, stop=True)
                        dd[t] = d
                    # apply phase (DVE)
                    for t in active:
                        ts = bass.ds(t * NTILE, NTILE)
                        loc = (t - c0) * NTILE
                        src = dd[t]
                        if use_bias:
                            nc.vector.tensor_scalar_add(src, src, b4r[:, 0:1])
                        if t in mks:
                            dm = tmpp.tile([4, NTILE], F32, tag="dm")
                            nc.vector.tensor_mul(dm, src, mks[t])
                            src = dm
                        if r + 1 < len(schedule[t]):
                            # fused f32 add + bf16 cast of the new state
                            nc.vector.tensor_add(stg[0:4, loc:loc + NTILE],
                                                 xs[:, ts], src)
                        nc.vector.tensor_add(xs[:, ts], xs[:, ts], src)
                if g0 == c0:
                    fetch_stg(ci + 1)   # prefetch during this chunk's tail
                    fetch_stg(ci + 2)
                g1 = min(g0 + GROUP, c1)
                nc.sync.dma_start(out=out_d[:, g0 * NTILE:g1 * NTILE],
                                  in_=xs[:, g0 * NTILE:g1 * NTILE])

    nc.compile()
    return nc


def kernel(state_dz, z_frac, W1, b1, W2, b2, W3, b3, W4, b4):
    global LAST_EXEC_NS
    import os
    import ml_dtypes

    state_dz = np.ascontiguousarray(state_dz, dtype=np.float32)
    z_frac = np.ascontiguousarray(z_frac, dtype=np.float32)
    W1 = np.asarray(W1, np.float32); W2 = np.asarray(W2, np.float32)
    W3 = np.asarray(W3, np.float32); W4 = np.asarray(W4, np.float32)
    b1 = np.asarray(b1, np.float32); b2 = np.asarray(b2, np.float32)
    b3 = np.asarray(b3, np.float32); b4 = np.asarray(b4, np.float32)

    B = state_dz.shape[0]
    assert B % (NCORES * NTILE) == 0, f"B={B} must be divisible by {NCORES * NTILE}"
    n_core = B // NCORES
    tiles = n_core // NTILE

    # ---- host-side derived quantities (bitwise-identical fp32 ops vs jax)
    dz_sub = (state_dz[:, 5] / np.float32(8.0)).astype(np.float32)
    cont = (z_frac * np.float32(NSTEPS)).astype(np.float32)
    n_full = np.floor(cont).astype(np.float32)
    frac = (cont - n_full).astype(np.float32)
    dz_part = (frac * dz_sub).astype(np.float32)
    has_part = (frac > np.float32(1e-6)).astype(np.float32)
    n_int = np.minimum(n_full, NSTEPS).astype(np.int64)

    # ---- sort desc by n_full, deal round-robin to cores
    order = np.argsort(-n_int, kind="stable")
    perms = [order[c::NCORES] for c in range(NCORES)]

    # ---- per-core input tensors (stat rows 0-3 seed the round-0 bf16 state)
    xss, sts = [], []
    for c in range(NCORES):
        p = perms[c]
        xss.append(np.ascontiguousarray(state_dz[p, 0:4].T, np.float32))
        st = np.zeros((8, n_core), np.float32)
        st[0:4] = state_dz[p, 0:4].T
        st[4] = state_dz[p, 4]
        st[5] = dz_sub[p]
        st[6] = dz_part[p]
        sts.append(st.astype(ml_dtypes.bfloat16))

    # ---- union schedule across cores (SPMD: one program for all cores)
    sched = []
    for t in range(tiles):
        sl = slice(t * NTILE, (t + 1) * NTILE)
        smax, smin = 0, NSTEPS
        anyp, allp = False, True
        for c in range(NCORES):
            nf = n_int[perms[c][sl]]
            smax = max(smax, int(nf.max()))
            smin = min(smin, int(nf.min()))
            hp = has_part[perms[c][sl]]
            anyp = anyp or bool(hp.any())
            allp = allp and bool(hp.all())
        evals = []
        for s in range(min(smax, NSTEPS)):
            evals.append((False, smin <= s))
        if anyp:
            evals.append((True, not allp))
        sched.append(tuple(evals))
    sched = tuple(sched)

    # masked evals in DEVICE EMISSION order; track each tile's step counter
    masked_evals = []
    step_no = [0] * tiles
    for t, (is_partial, use_mask) in _emit_order(sched, tiles):
        s = None if is_partial else step_no[t]
        if not is_partial:
            step_no[t] += 1
        if use_mask:
            masked_evals.append((t, s))

    # ---- packed mask rows (4x replicated), one NTILE slot per masked eval
    nm = max(1, len(masked_evals))
    maskcats = [np.zeros((4, nm * NTILE), np.float32) for _ in range(NCORES)]
    for j, (t, s) in enumerate(masked_evals):
        sl = slice(t * NTILE, (t + 1) * NTILE)
        for c in range(NCORES):
            idx = perms[c][sl]
            row = has_part[idx] if s is None else (n_full[idx] > s).astype(np.float32)
            maskcats[c][:, j * NTILE:(j + 1) * NTILE] = row[None, :]

    use_bias = bool(np.any(b1) or np.any(b2) or np.any(b3) or np.any(b4))

    key = (sched, use_bias, n_core)
    if key not in _BUILD_CACHE:
        _BUILD_CACHE[key] = _build(sched, use_bias, n_core)
    nc = _BUILD_CACHE[key]

    # ---- weight tensors in lhsT layouts, bf16
    w1h = np.zeros((8, 512), np.float32)
    w1h[0:6, 0:256] = W1                      # full: state,qop,dz_sub
    w1h[0:5, 256:512] = W1[0:5]               # partial: dz slot zeroed,
    w1h[6, 256:512] = W1[5]                   # dz weight reads dz_partial row
    w2h = np.concatenate([W2[0:128], W2[128:256]], axis=1)
    w3h = np.concatenate([W3[0:128], W3[128:256]], axis=1)
    w4h = np.concatenate([W4[0:128], W4[128:256]], axis=1)

    bf = ml_dtypes.bfloat16
    in_map = {"w1": w1h.astype(bf), "w2": w2h.astype(bf),
              "w3": w3h.astype(bf), "w4": w4h.astype(bf)}
    if use_bias:
        b123 = np.stack([b1[0:128], b1[128:256], b2[0:128], b2[128:256],
                         b3[0:128], b3[128:256]], axis=1).astype(np.float32)
        in_map["b123"] = b123
        in_map["b4r"] = b4.reshape(4, 1).astype(np.float32)

    in_maps = [{**in_map, "xs": xss[c], "stat": sts[c], "maskcat": maskcats[c]}
               for c in range(NCORES)]

    trace = os.environ.get("BASSK_TRACE") == "1" and _install_ntff_hook()
    try:
        res = run_bass_kernel_spmd(nc, in_maps, list(range(NCORES)), trace=trace)
    except Exception:
        if not trace:
            raise
        res = run_bass_kernel_spmd(nc, in_maps, list(range(NCORES)), trace=False)
    LAST_EXEC_NS = res.exec_time_ns

    out = np.empty((B, 4), np.float32)
    for c in range(NCORES):
        out[perms[c], :] = res.results[c]["outT"].T
    return out
